# revision 27
# baseline (speedup 1.0000x reference)
"""DGCN diffusion-graph-conv kernel for 8 Trainium2 NeuronCores.

Math (per batch b):
    x_cat = concat(inputs, state_t, ones)      # [N, C+1]  (ones row folds bias)
    out_b = tanh( x_cat @ W0' + sum_s [A_s @ Y1s + B_s @ (2*Y2s)] )
  where (projection-first reformulation + host-precomputed Chebyshev square):
    W0'  = W_m0 - W_m2 - W_m4 (+ bias row)     # folds the "-x0" Chebyshev terms
    Y1s  = x_cat @ W_{2s+1},  Y2s = x_cat @ W_{2s+2}     # [N, HID]
    B_s  = A_s @ A_s                            # entries k/256: exact in fp8e4

Distribution: pure data-parallel over batch (2 batches per core, 8 cores),
no collectives.

Device dataflow (fp8 DoubleRow, feature-major spmm):
  - All four sparse matrices (A_0, A_1, B_0, B_1) are densified on the host
    into one DoubleRow-friendly fp8 layout
    ablk[j, p, m, k, i] = M_m[i, (2p+k)*128 + j]; fp8e4 is exact for both
    k/16 (A) and k/256 (B) entries, so only the Y projections carry
    quantization noise (~7e-3 rel overall).
  - One projection phase: psum[node-tile, 512] = x_catT @ [W1_0 W1_1
    2W2_0 2W2_2], cast once per (b, t) to the fp8 stationary yb[b].
  - One mega-pass accumulates, per PSUM group [h, 512 nodes]:
    z0 (bf16 MMs) + 4 sweeps of 16 DoubleRow MMs (stationary = yb pair
    block, moving = ablk tile, FD=512, 2x PE throughput), then drains
    straight through tanh to the feature-major output (host transposes).
  - PE warm-up MMs run during the input-DMA prologue to lift the HAM
    clock-gate to 2.4GHz before real work; A-tile DMA triggers alternate
    between the Sync and Scalar hardware DGE queues to halve trigger
    latency.
"""

import numpy as np

import concourse.bass as bass
import concourse.bacc as bacc
import concourse.tile as tile
from concourse import mybir
from concourse.bass import ts
from concourse.bass_utils import run_bass_kernel_spmd

F32 = mybir.dt.float32
BF16 = mybir.dt.bfloat16
FP8 = mybir.dt.float8e4
Alu = mybir.AluOpType
Act = mybir.ActivationFunctionType
DR = mybir.MatmulPerfMode.DoubleRow

B, N, IN_DIM, HID = 16, 4096, 64, 128
C = IN_DIM + HID              # 192
CB = C + 1                    # +1 ones row (bias folding)
M = 5
DEG = 16
N_CORES = 8
BL = B // N_CORES             # 2 batches per core
N_SUP = 2
NT = N // 128                 # 32 node tiles
NJP = NT // 2                 # 16 node-tile pairs (DoubleRow contraction)
NC4 = N // 4                  # 1024: i-chunk (4 PSUM groups live)
NM = 2 * N_SUP                # 4 sweep matrices: A_0, A_1, B_0, B_1

_prog_cache: dict = {}


def _install_ntff_hook():
    """Benchmark-only: wire up the NTFF profile hook that bass_utils
    expects under axon when trace=True (the antenv.axon_hooks shim module
    is absent in this image), and stub out the S3 artifact upload."""
    import sys
    import types

    try:
        import antenv
        import concourse.bass_utils as bu

        bu.upload_artifacts = lambda tmpdir: "local://" + tmpdir
        if "antenv.axon_hooks" in sys.modules:
            return
        import trn_agent_boot.trn_boot as tb

        hook = tb._ntff_profile_via_ctypes("/opt/axon/libaxon_pjrt.so")
        mod = types.ModuleType("antenv.axon_hooks")
        mod.get_axon_ntff_profile_hook = lambda: hook
        mod.set_axon_ntff_profile_hook = lambda h: None
        sys.modules["antenv.axon_hooks"] = mod
        antenv.axon_hooks = mod
    except Exception as e:  # profiling is best-effort
        print(f"ntff hook install failed: {e}")


def _build_program(n_sup: int):
    nc = bacc.Bacc(
        "TRN2",
        target_bir_lowering=False,
        debug=False,
        enable_asserts=False,
        num_devices=N_CORES,
    )
    nm = 2 * n_sup

    x0T_d = nc.dram_tensor("x0T", [BL, CB, N], BF16, kind="ExternalInput").ap()
    # fp8 DR-packed x_cat^T (c padded to 2x128) and Y-projection weights:
    # x8p[b, cc, kt, n] = x_cat^T[kt*128+cc, n]; wc8[cc, kt, col] likewise
    x8p_d = nc.dram_tensor("x8p", [BL, 128, 2, N], FP8, kind="ExternalInput").ap()
    wc8_d = nc.dram_tensor("wc8", [128, 2, 512], FP8, kind="ExternalInput").ap()
    # wc cols: 0:128 W0'+bias, 128:256 W1_0, 256:384 W1_1, 384:512 2*W2_0,
    # 512:640 2*W2_1
    wc_d = nc.dram_tensor("wc", [CB, M * HID], F32, kind="ExternalInput").ap()
    # ablk[j, p, m, k, i] = M_m[i, (2p+k)*128 + j], M = [A_0, A_1, B_0, B_1]
    ablk_d = nc.dram_tensor(
        "ablk", [128, NJP, nm, 2, N], FP8, kind="ExternalInput"
    ).ap()
    ident_d = nc.dram_tensor("ident", [128, 128], BF16, kind="ExternalInput").ap()
    # feature-major output: out[b, h, n]
    out_d = nc.dram_tensor("out", [BL, 128, N], F32, kind="ExternalOutput").ap()

    KCH = [(0, 128), (128, CB - 128)]   # C+1 split into partition chunks
    kn1 = CB - 128

    with tile.TileContext(nc) as tc:
        with (
            tc.tile_pool(name="persist", bufs=1) as persist,
            tc.tile_pool(name="apool", bufs=28) as apool,
            tc.tile_pool(name="stage", bufs=4) as stage,
            tc.tile_pool(name="psA", bufs=6, space="PSUM") as psA,
            tc.tile_pool(name="psT", bufs=2, space="PSUM") as psT,
        ):
            # ---------- identity + weights first, then PE warm-up ----------
            ident = persist.tile([128, 128], BF16, tag="ident")
            nc.sync.dma_start(out=ident[:], in_=ident_d[:, :])
            # early warm-up needs only ident (32KB DMA)
            for w in range(12):
                wp = psT.tile([128, 512], F32, tag="psT", name=f"warme{w}")
                nc.tensor.matmul(wp[:, 0:128], lhsT=ident[:], rhs=ident[:],
                                 start=True, stop=True)
            wc_bf = []
            for kc, (k0, kn) in enumerate(KCH):
                wst = stage.tile([128, M * HID], F32, tag="stage", name=f"wst{kc}")
                nc.sync.dma_start(out=wst[:kn, :], in_=wc_d[k0 : k0 + kn, :])
                wb = persist.tile([128, M * HID], BF16, tag=f"wc{kc}")
                nc.scalar.copy(out=wb[:kn, :], in_=wst[:kn, :])
                wc_bf.append(wb)
            # keep PE busy (and the HAM clock-gate warming) through the
            # rest of the input-DMA prologue
            for w in range(36):
                wp = psT.tile([128, 512], F32, tag="psT", name=f"warm{w}")
                nc.tensor.matmul(wp[:], lhsT=ident[:], rhs=wc_bf[0][:, 0:512],
                                 start=True, stop=True)
            # fp8 DR-packed projection operands
            wc8 = persist.tile([128, 2, 512], FP8, tag="wc8")
            nc.sync.dma_start(out=wc8[:], in_=wc8_d[:, :, :])

            # ---------- load x0T (bf16 for z0) + x8p (fp8 for Y projs) ----
            # x0T_bf[b]: [128, 8192]; cols [0:N] = c-chunk 0, [N:2N] = chunk 1
            x0T_bf, x8p = [], []
            for b in range(BL):
                x8b = persist.tile([128, 2, N], FP8, tag=f"x8_{b}", name=f"x8_{b}")
                nc.sync.dma_start(out=x8b[:], in_=x8p_d[b])
                x8p.append(x8b)
                xb = persist.tile([128, 2 * N], BF16, tag=f"xb{b}")
                for half in range(2):
                    sl = ts(half, N // 2)
                    nc.sync.dma_start(out=xb[:, sl], in_=x0T_d[b, 0:128, sl])
                    nc.sync.dma_start(
                        out=xb[:kn1, N + half * (N // 2) : N + (half + 1) * (N // 2)],
                        in_=x0T_d[b, 128:CB, sl],
                    )
                x0T_bf.append(xb)

            # ---------- stationaries ----------
            # yb[b][j, jb, m*128+h] = fp8 of [Y1_0 | Y1_1 | 2Y2_0 | 2Y2_1]
            yb = [persist.tile([128, NT, nm * HID], FP8, tag=f"yb{b}",
                               name=f"yb{b}") for b in range(BL)]

            # ---------- projections (one fp8 DoubleRow MM per tile) -------
            for b in range(BL):
                for t in range(NT):
                    p = psA.tile([128, 512], F32, tag="psA", name=f"yp{b}_{t}")
                    nc.tensor.matmul(
                        p[:],
                        lhsT=x8p[b][:, :, t * 128 : (t + 1) * 128],
                        rhs=wc8[:],
                        start=True,
                        stop=True,
                        perf_mode=DR,
                    )
                    if t & 1:
                        nc.vector.tensor_copy(out=yb[b][:, t, :], in_=p[:])
                    else:
                        nc.scalar.copy(out=yb[b][:, t, :], in_=p[:])

            # ---------- mega-pass: tanh(z0 + sum_m M_m @ Y_m) ----------
            for chunk in range(4):
                psums = []
                for g in range(4):
                    isup, fc = g >> 1, g & 1
                    n0 = chunk * NC4 + isup * 512
                    pt = psA.tile([128, 512], F32, tag="psA",
                                  name=f"mp_{chunk}_{g}")
                    psums.append((pt, isup, fc, n0))
                    for kc, (k0, kn) in enumerate(KCH):
                        nc.tensor.matmul(
                            pt[:],
                            lhsT=wc_bf[kc][:kn, 0:128],
                            rhs=x0T_bf[fc][:kn, kc * N + n0 : kc * N + n0 + 512],
                            start=(kc == 0),
                            stop=False,
                        )
                for m in range(nm):
                    for p in range(NJP):
                        at = apool.tile([128, 2, NC4], FP8, tag="apool",
                                        name=f"a_{chunk}_{m}_{p}")
                        eng = nc.sync if (p & 1) == 0 else nc.scalar
                        eng.dma_start(
                            out=at[:],
                            in_=ablk_d[:, p, m, :, chunk * NC4 : (chunk + 1) * NC4],
                        )
                        for pt, isup, fc, n0 in psums:
                            nc.tensor.matmul(
                                pt[:],
                                lhsT=yb[fc][:, 2 * p : 2 * p + 2, ts(m, HID)],
                                rhs=at[:, :, isup * 512 : isup * 512 + 512],
                                start=False,
                                stop=(m == nm - 1 and p == NJP - 1),
                                perf_mode=DR,
                            )
                for pt, isup, fc, n0 in psums:
                    # DVE copy frees the PSUM bank immediately (the scalar
                    # queue is backed up with DMA triggers); tanh runs later
                    # off SBUF.
                    st = stage.tile([128, M * HID], F32, tag="stage",
                                    name=f"st{fc}_{n0}")
                    nc.vector.tensor_copy(out=st[:, 0:512], in_=pt[:])
                    ot = stage.tile([128, M * HID], F32, tag="stage",
                                    name=f"ot{fc}_{n0}")
                    nc.scalar.activation(out=ot[:, 0:512], in_=st[:, 0:512],
                                         func=Act.Tanh)
                    nc.sync.dma_start(out=out_d[fc, :, n0 : n0 + 512], in_=ot[:, 0:512])

    nc.compile()
    return nc


def _build_ablk(sup_rows, sup_cols, sup_vals, n_sup):
    """Densify A_s and B_s = A_s^2 into the DoubleRow moving layout.

    ablk[j, p, m, k, i] = M_m[i, (2p+k)*128 + j]; fp8e4 exact for both.
    """
    import ml_dtypes
    from scipy import sparse

    out = np.empty((128, NJP, 2 * n_sup, 2, N), dtype=ml_dtypes.float8_e4m3)
    for s in range(n_sup):
        a_sp = sparse.csr_matrix(
            (
                sup_vals[s].astype(np.float32),
                (sup_rows[s].astype(np.int64), sup_cols[s].astype(np.int64)),
            ),
            shape=(N, N),
        )
        a = a_sp.toarray()
        b2 = (a_sp @ a_sp).toarray()
        for m, mat in ((s, a), (n_sup + s, b2)):
            # at[col, row] layout -> [j, p, k, i]
            out[:, :, m] = (
                mat.T.reshape(NJP, 2, 128, N).transpose(2, 0, 1, 3)
                .astype(ml_dtypes.float8_e4m3)
            )
    return out


def _prep_core_inputs(inputs, state_t, weights, biases, sup_rows, sup_cols, sup_vals):
    """Host-side sharding: batch-parallel slices + layout prep."""
    import ml_dtypes

    w5 = weights.reshape(C, M, HID)
    wc = np.zeros((CB, M, HID), dtype=np.float32)
    wc[:C, 0] = w5[:, 0] - w5[:, 2] - w5[:, 4]
    wc[C, 0] = biases.astype(np.float32)          # bias via ones row
    wc[:C, 1] = w5[:, 1]                          # W1_0
    wc[:C, 2] = w5[:, 3]                          # W1_1
    wc[:C, 3] = 2.0 * w5[:, 2]                    # 2*W2_0
    wc[:C, 4] = 2.0 * w5[:, 4]                    # 2*W2_1
    wc = np.ascontiguousarray(wc.reshape(CB, M * HID))

    ablk = _build_ablk(sup_rows, sup_cols, sup_vals, N_SUP)
    ident = np.eye(128, dtype=ml_dtypes.bfloat16)
    # wc8[cc, kt, col] = wc[kt*128+cc, 128+col] (fp8, c padded to 256)
    wcp = np.zeros((2, 128, 512), dtype=np.float32)
    wcp[0] = wc[0:128, 128:640]
    wcp[1, : CB - 128] = wc[128:CB, 128:640]
    wc8 = np.ascontiguousarray(wcp.transpose(1, 0, 2)).astype(ml_dtypes.float8_e4m3)

    in_maps = []
    for core in range(N_CORES):
        b0 = core * BL
        xcat = np.concatenate(
            [
                inputs[b0 : b0 + BL],
                state_t[b0 : b0 + BL],
                np.ones((BL, N, 1), dtype=np.float32),
            ],
            axis=2,
        )  # [BL, N, CB]
        x0T = np.ascontiguousarray(xcat.transpose(0, 2, 1)).astype(ml_dtypes.bfloat16)
        # x8p[b, cc, kt, n] = x_cat^T[kt*128+cc, n] (fp8, c padded to 256)
        xp = np.zeros((BL, 2, 128, N), dtype=np.float32)
        xp[:, 0] = xcat.transpose(0, 2, 1)[:, 0:128]
        xp[:, 1, : CB - 128] = xcat.transpose(0, 2, 1)[:, 128:CB]
        x8p = np.ascontiguousarray(xp.transpose(0, 2, 1, 3)).astype(
            ml_dtypes.float8_e4m3
        )
        in_maps.append(
            {"x0T": x0T, "wc": wc, "ablk": ablk, "ident": ident,
             "x8p": x8p, "wc8": wc8}
        )
    return in_maps


def kernel(
    inputs,
    state_t,
    weights,
    biases,
    sup_rows,
    sup_cols,
    sup_vals,
    _bench=None,
):
    inputs = np.asarray(inputs)
    state_t = np.asarray(state_t)
    weights = np.asarray(weights, dtype=np.float32)
    biases = np.asarray(biases, dtype=np.float32)
    sup_rows = np.asarray(sup_rows)
    sup_cols = np.asarray(sup_cols)
    sup_vals = np.asarray(sup_vals)

    if "prog" not in _prog_cache:
        _prog_cache["prog"] = _build_program(N_SUP)
    nc = _prog_cache["prog"]

    in_maps = _prep_core_inputs(
        inputs, state_t, weights, biases, sup_rows, sup_cols, sup_vals
    )
    trace = _bench is not None
    if trace:
        _install_ntff_hook()
    res = run_bass_kernel_spmd(nc, in_maps, list(range(N_CORES)), trace=trace)
    if _bench is not None:
        _bench["exec_time_ns"] = res.exec_time_ns
        _bench["mean_exec_time_ns"] = res.mean_exec_time_ns
        _bench["results"] = res

    out = np.empty((B, N, HID), dtype=np.float32)
    for core in range(N_CORES):
        o = res.results[core]["out"]  # [BL, 128, N] feature-major
        for b in range(BL):
            out[core * BL + b] = np.asarray(o[b], dtype=np.float32).T
    return out


# revision 28
# speedup vs baseline: 1.0113x; 1.0113x over previous
"""DGCN diffusion-graph-conv kernel for 8 Trainium2 NeuronCores.

Math (per batch b):
    x_cat = concat(inputs, state_t, ones)      # [N, C+1]  (ones row folds bias)
    out_b = tanh( x_cat @ W0' + sum_s [A_s @ Y1s + B_s @ (2*Y2s)] )
  where (projection-first reformulation + host-precomputed Chebyshev square):
    W0'  = W_m0 - W_m2 - W_m4 (+ bias row)     # folds the "-x0" Chebyshev terms
    Y1s  = x_cat @ W_{2s+1},  Y2s = x_cat @ W_{2s+2}     # [N, HID]
    B_s  = A_s @ A_s                            # entries k/256: exact in fp8e4

Distribution: pure data-parallel over batch (2 batches per core, 8 cores),
no collectives.

Device dataflow (fp8 DoubleRow, feature-major spmm):
  - All four sparse matrices (A_0, A_1, B_0, B_1) are densified on the host
    into one DoubleRow-friendly fp8 layout
    ablk[j, p, m, k, i] = M_m[i, (2p+k)*128 + j]; fp8e4 is exact for both
    k/16 (A) and k/256 (B) entries, so only the Y projections carry
    quantization noise (~7e-3 rel overall).
  - One projection phase: psum[node-tile, 512] = x_catT @ [W1_0 W1_1
    2W2_0 2W2_2], cast once per (b, t) to the fp8 stationary yb[b].
  - One mega-pass accumulates, per PSUM group [h, 512 nodes]:
    z0 (bf16 MMs) + 4 sweeps of 16 DoubleRow MMs (stationary = yb pair
    block, moving = ablk tile, FD=512, 2x PE throughput), then drains
    straight through tanh to the feature-major output (host transposes).
  - PE warm-up MMs run during the input-DMA prologue to lift the HAM
    clock-gate to 2.4GHz before real work; A-tile DMA triggers alternate
    between the Sync and Scalar hardware DGE queues to halve trigger
    latency.
"""

import numpy as np

import concourse.bass as bass
import concourse.bacc as bacc
import concourse.tile as tile
from concourse import mybir
from concourse.bass import ts
from concourse.bass_utils import run_bass_kernel_spmd

F32 = mybir.dt.float32
BF16 = mybir.dt.bfloat16
FP8 = mybir.dt.float8e4
Alu = mybir.AluOpType
Act = mybir.ActivationFunctionType
DR = mybir.MatmulPerfMode.DoubleRow

B, N, IN_DIM, HID = 16, 4096, 64, 128
C = IN_DIM + HID              # 192
CB = C + 1                    # +1 ones row (bias folding)
M = 5
DEG = 16
N_CORES = 8
BL = B // N_CORES             # 2 batches per core
N_SUP = 2
NT = N // 128                 # 32 node tiles
NJP = NT // 2                 # 16 node-tile pairs (DoubleRow contraction)
NC4 = N // 4                  # 1024: i-chunk (4 PSUM groups live)
NM = 2 * N_SUP                # 4 sweep matrices: A_0, A_1, B_0, B_1

_prog_cache: dict = {}


def _install_ntff_hook():
    """Benchmark-only: wire up the NTFF profile hook that bass_utils
    expects under axon when trace=True (the antenv.axon_hooks shim module
    is absent in this image), and stub out the S3 artifact upload."""
    import sys
    import types

    try:
        import antenv
        import concourse.bass_utils as bu

        bu.upload_artifacts = lambda tmpdir: "local://" + tmpdir
        if "antenv.axon_hooks" in sys.modules:
            return
        import trn_agent_boot.trn_boot as tb

        hook = tb._ntff_profile_via_ctypes("/opt/axon/libaxon_pjrt.so")
        mod = types.ModuleType("antenv.axon_hooks")
        mod.get_axon_ntff_profile_hook = lambda: hook
        mod.set_axon_ntff_profile_hook = lambda h: None
        sys.modules["antenv.axon_hooks"] = mod
        antenv.axon_hooks = mod
    except Exception as e:  # profiling is best-effort
        print(f"ntff hook install failed: {e}")


def _build_program(n_sup: int):
    nc = bacc.Bacc(
        "TRN2",
        target_bir_lowering=False,
        debug=False,
        enable_asserts=False,
        num_devices=N_CORES,
    )
    nm = 2 * n_sup

    x0T_d = nc.dram_tensor("x0T", [BL, CB, N], BF16, kind="ExternalInput").ap()
    # fp8 DR-packed x_cat^T (c padded to 2x128) and Y-projection weights:
    # x8p[b, cc, kt, n] = x_cat^T[kt*128+cc, n]; wc8[cc, kt, col] likewise
    x8p_d = nc.dram_tensor("x8p", [BL, 128, 2, N], FP8, kind="ExternalInput").ap()
    wc8_d = nc.dram_tensor("wc8", [128, 2, 512], FP8, kind="ExternalInput").ap()
    # wc cols: 0:128 W0'+bias, 128:256 W1_0, 256:384 W1_1, 384:512 2*W2_0,
    # 512:640 2*W2_1
    wc_d = nc.dram_tensor("wc", [CB, M * HID], F32, kind="ExternalInput").ap()
    # ablk[j, p, m, k, i] = M_m[i, (2p+k)*128 + j], M = [A_0, A_1, B_0, B_1]
    ablk_d = nc.dram_tensor(
        "ablk", [128, NJP, nm, 2, N], FP8, kind="ExternalInput"
    ).ap()
    ident_d = nc.dram_tensor("ident", [128, 128], BF16, kind="ExternalInput").ap()
    # feature-major output: out[b, h, n]
    out_d = nc.dram_tensor("out", [BL, 128, N], F32, kind="ExternalOutput").ap()

    KCH = [(0, 128), (128, CB - 128)]   # C+1 split into partition chunks
    kn1 = CB - 128

    with tile.TileContext(nc) as tc:
        with (
            tc.tile_pool(name="persist", bufs=1) as persist,
            tc.tile_pool(name="apool", bufs=16) as apool,
            tc.tile_pool(name="stage", bufs=4) as stage,
            tc.tile_pool(name="psA", bufs=6, space="PSUM") as psA,
            tc.tile_pool(name="psT", bufs=2, space="PSUM") as psT,
        ):
            # ---------- identity + weights first, then PE warm-up ----------
            ident = persist.tile([128, 128], BF16, tag="ident")
            nc.sync.dma_start(out=ident[:], in_=ident_d[:, :])
            # early warm-up needs only ident (32KB DMA)
            for w in range(12):
                wp = psT.tile([128, 512], F32, tag="psT", name=f"warme{w}")
                nc.tensor.matmul(wp[:, 0:128], lhsT=ident[:], rhs=ident[:],
                                 start=True, stop=True)
            wc_bf = []
            for kc, (k0, kn) in enumerate(KCH):
                wst = stage.tile([128, M * HID], F32, tag="stage", name=f"wst{kc}")
                nc.sync.dma_start(out=wst[:kn, :], in_=wc_d[k0 : k0 + kn, :])
                wb = persist.tile([128, M * HID], BF16, tag=f"wc{kc}")
                nc.scalar.copy(out=wb[:kn, :], in_=wst[:kn, :])
                wc_bf.append(wb)
            # keep PE busy (and the HAM clock-gate warming) through the
            # rest of the input-DMA prologue
            for w in range(36):
                wp = psT.tile([128, 512], F32, tag="psT", name=f"warm{w}")
                nc.tensor.matmul(wp[:], lhsT=ident[:], rhs=wc_bf[0][:, 0:512],
                                 start=True, stop=True)
            # fp8 DR-packed projection operands
            wc8 = persist.tile([128, 2, 512], FP8, tag="wc8")
            nc.sync.dma_start(out=wc8[:], in_=wc8_d[:, :, :])

            # ---------- load x0T (bf16 for z0) + x8p (fp8 for Y projs) ----
            # x0T_bf[b]: [128, 8192]; cols [0:N] = c-chunk 0, [N:2N] = chunk 1
            x0T_bf, x8p = [], []
            for b in range(BL):
                x8b = persist.tile([128, 2, N], FP8, tag=f"x8_{b}", name=f"x8_{b}")
                nc.sync.dma_start(out=x8b[:], in_=x8p_d[b])
                x8p.append(x8b)
                xb = persist.tile([128, 2 * N], BF16, tag=f"xb{b}")
                for half in range(2):
                    sl = ts(half, N // 2)
                    nc.sync.dma_start(out=xb[:, sl], in_=x0T_d[b, 0:128, sl])
                    nc.sync.dma_start(
                        out=xb[:kn1, N + half * (N // 2) : N + (half + 1) * (N // 2)],
                        in_=x0T_d[b, 128:CB, sl],
                    )
                x0T_bf.append(xb)

            # ---------- stationaries ----------
            # yb[b][j, jb, m*128+h] = fp8 of [Y1_0 | Y1_1 | 2Y2_0 | 2Y2_1]
            yb = [persist.tile([128, NT, nm * HID], FP8, tag=f"yb{b}",
                               name=f"yb{b}") for b in range(BL)]

            # ---------- projections (one fp8 DoubleRow MM per tile) -------
            for b in range(BL):
                for t in range(NT):
                    p = psA.tile([128, 512], F32, tag="psA", name=f"yp{b}_{t}")
                    nc.tensor.matmul(
                        p[:],
                        lhsT=x8p[b][:, :, t * 128 : (t + 1) * 128],
                        rhs=wc8[:],
                        start=True,
                        stop=True,
                        perf_mode=DR,
                    )
                    if t & 1:
                        nc.vector.tensor_copy(out=yb[b][:, t, :], in_=p[:])
                    else:
                        nc.scalar.copy(out=yb[b][:, t, :], in_=p[:])

            # ---------- mega-pass: tanh(z0 + sum_m M_m @ Y_m) ----------
            for chunk in range(4):
                psums = []
                for g in range(4):
                    isup, fc = g >> 1, g & 1
                    n0 = chunk * NC4 + isup * 512
                    pt = psA.tile([128, 512], F32, tag="psA",
                                  name=f"mp_{chunk}_{g}")
                    psums.append((pt, isup, fc, n0))
                    for kc, (k0, kn) in enumerate(KCH):
                        nc.tensor.matmul(
                            pt[:],
                            lhsT=wc_bf[kc][:kn, 0:128],
                            rhs=x0T_bf[fc][:kn, kc * N + n0 : kc * N + n0 + 512],
                            start=(kc == 0),
                            stop=False,
                        )
                for m in range(nm):
                    for p in range(NJP):
                        at = apool.tile([128, 2, NC4], FP8, tag="apool",
                                        name=f"a_{chunk}_{m}_{p}")
                        eng = nc.sync if (p & 1) == 0 else nc.scalar
                        eng.dma_start(
                            out=at[:],
                            in_=ablk_d[:, p, m, :, chunk * NC4 : (chunk + 1) * NC4],
                        )
                        for pt, isup, fc, n0 in psums:
                            nc.tensor.matmul(
                                pt[:],
                                lhsT=yb[fc][:, 2 * p : 2 * p + 2, ts(m, HID)],
                                rhs=at[:, :, isup * 512 : isup * 512 + 512],
                                start=False,
                                stop=(m == nm - 1 and p == NJP - 1),
                                perf_mode=DR,
                            )
                for pt, isup, fc, n0 in psums:
                    # DVE copy frees the PSUM bank immediately (the scalar
                    # queue is backed up with DMA triggers); tanh runs later
                    # off SBUF.
                    st = stage.tile([128, M * HID], F32, tag="stage",
                                    name=f"st{fc}_{n0}")
                    nc.vector.tensor_copy(out=st[:, 0:512], in_=pt[:])
                    ot = stage.tile([128, M * HID], F32, tag="stage",
                                    name=f"ot{fc}_{n0}")
                    nc.scalar.activation(out=ot[:, 0:512], in_=st[:, 0:512],
                                         func=Act.Tanh)
                    nc.sync.dma_start(out=out_d[fc, :, n0 : n0 + 512], in_=ot[:, 0:512])

    nc.compile()
    return nc


def _build_ablk(sup_rows, sup_cols, sup_vals, n_sup):
    """Densify A_s and B_s = A_s^2 into the DoubleRow moving layout.

    ablk[j, p, m, k, i] = M_m[i, (2p+k)*128 + j]; fp8e4 exact for both.
    """
    import ml_dtypes
    from scipy import sparse

    out = np.empty((128, NJP, 2 * n_sup, 2, N), dtype=ml_dtypes.float8_e4m3)
    for s in range(n_sup):
        a_sp = sparse.csr_matrix(
            (
                sup_vals[s].astype(np.float32),
                (sup_rows[s].astype(np.int64), sup_cols[s].astype(np.int64)),
            ),
            shape=(N, N),
        )
        a = a_sp.toarray()
        b2 = (a_sp @ a_sp).toarray()
        for m, mat in ((s, a), (n_sup + s, b2)):
            # at[col, row] layout -> [j, p, k, i]
            out[:, :, m] = (
                mat.T.reshape(NJP, 2, 128, N).transpose(2, 0, 1, 3)
                .astype(ml_dtypes.float8_e4m3)
            )
    return out


def _prep_core_inputs(inputs, state_t, weights, biases, sup_rows, sup_cols, sup_vals):
    """Host-side sharding: batch-parallel slices + layout prep."""
    import ml_dtypes

    w5 = weights.reshape(C, M, HID)
    wc = np.zeros((CB, M, HID), dtype=np.float32)
    wc[:C, 0] = w5[:, 0] - w5[:, 2] - w5[:, 4]
    wc[C, 0] = biases.astype(np.float32)          # bias via ones row
    wc[:C, 1] = w5[:, 1]                          # W1_0
    wc[:C, 2] = w5[:, 3]                          # W1_1
    wc[:C, 3] = 2.0 * w5[:, 2]                    # 2*W2_0
    wc[:C, 4] = 2.0 * w5[:, 4]                    # 2*W2_1
    wc = np.ascontiguousarray(wc.reshape(CB, M * HID))

    ablk = _build_ablk(sup_rows, sup_cols, sup_vals, N_SUP)
    ident = np.eye(128, dtype=ml_dtypes.bfloat16)
    # wc8[cc, kt, col] = wc[kt*128+cc, 128+col] (fp8, c padded to 256)
    wcp = np.zeros((2, 128, 512), dtype=np.float32)
    wcp[0] = wc[0:128, 128:640]
    wcp[1, : CB - 128] = wc[128:CB, 128:640]
    wc8 = np.ascontiguousarray(wcp.transpose(1, 0, 2)).astype(ml_dtypes.float8_e4m3)

    in_maps = []
    for core in range(N_CORES):
        b0 = core * BL
        xcat = np.concatenate(
            [
                inputs[b0 : b0 + BL],
                state_t[b0 : b0 + BL],
                np.ones((BL, N, 1), dtype=np.float32),
            ],
            axis=2,
        )  # [BL, N, CB]
        x0T = np.ascontiguousarray(xcat.transpose(0, 2, 1)).astype(ml_dtypes.bfloat16)
        # x8p[b, cc, kt, n] = x_cat^T[kt*128+cc, n] (fp8, c padded to 256)
        xp = np.zeros((BL, 2, 128, N), dtype=np.float32)
        xp[:, 0] = xcat.transpose(0, 2, 1)[:, 0:128]
        xp[:, 1, : CB - 128] = xcat.transpose(0, 2, 1)[:, 128:CB]
        x8p = np.ascontiguousarray(xp.transpose(0, 2, 1, 3)).astype(
            ml_dtypes.float8_e4m3
        )
        in_maps.append(
            {"x0T": x0T, "wc": wc, "ablk": ablk, "ident": ident,
             "x8p": x8p, "wc8": wc8}
        )
    return in_maps


def kernel(
    inputs,
    state_t,
    weights,
    biases,
    sup_rows,
    sup_cols,
    sup_vals,
    _bench=None,
):
    inputs = np.asarray(inputs)
    state_t = np.asarray(state_t)
    weights = np.asarray(weights, dtype=np.float32)
    biases = np.asarray(biases, dtype=np.float32)
    sup_rows = np.asarray(sup_rows)
    sup_cols = np.asarray(sup_cols)
    sup_vals = np.asarray(sup_vals)

    if "prog" not in _prog_cache:
        _prog_cache["prog"] = _build_program(N_SUP)
    nc = _prog_cache["prog"]

    in_maps = _prep_core_inputs(
        inputs, state_t, weights, biases, sup_rows, sup_cols, sup_vals
    )
    trace = _bench is not None
    if trace:
        _install_ntff_hook()
    res = run_bass_kernel_spmd(nc, in_maps, list(range(N_CORES)), trace=trace)
    if _bench is not None:
        _bench["exec_time_ns"] = res.exec_time_ns
        _bench["mean_exec_time_ns"] = res.mean_exec_time_ns
        _bench["results"] = res

    out = np.empty((B, N, HID), dtype=np.float32)
    for core in range(N_CORES):
        o = res.results[core]["out"]  # [BL, 128, N] feature-major
        for b in range(BL):
            out[core * BL + b] = np.asarray(o[b], dtype=np.float32).T
    return out


# revision 29
# speedup vs baseline: 1.0151x; 1.0038x over previous
"""DGCN diffusion-graph-conv kernel for 8 Trainium2 NeuronCores.

Math (per batch b):
    x_cat = concat(inputs, state_t, ones)      # [N, C+1]  (ones row folds bias)
    out_b = tanh( x_cat @ W0' + sum_s [A_s @ Y1s + B_s @ (2*Y2s)] )
  where (projection-first reformulation + host-precomputed Chebyshev square):
    W0'  = W_m0 - W_m2 - W_m4 (+ bias row)     # folds the "-x0" Chebyshev terms
    Y1s  = x_cat @ W_{2s+1},  Y2s = x_cat @ W_{2s+2}     # [N, HID]
    B_s  = A_s @ A_s                            # entries k/256: exact in fp8e4

Distribution: pure data-parallel over batch (2 batches per core, 8 cores),
no collectives.

Device dataflow (fp8 DoubleRow, feature-major spmm):
  - All four sparse matrices (A_0, A_1, B_0, B_1) are densified on the host
    into one DoubleRow-friendly fp8 layout
    ablk[j, p, m, k, i] = M_m[i, (2p+k)*128 + j]; fp8e4 is exact for both
    k/16 (A) and k/256 (B) entries, so only the Y projections carry
    quantization noise (~7e-3 rel overall).
  - One projection phase: psum[node-tile, 512] = x_catT @ [W1_0 W1_1
    2W2_0 2W2_2], cast once per (b, t) to the fp8 stationary yb[b].
  - One mega-pass accumulates, per PSUM group [h, 512 nodes]:
    z0 (bf16 MMs) + 4 sweeps of 16 DoubleRow MMs (stationary = yb pair
    block, moving = ablk tile, FD=512, 2x PE throughput), then drains
    straight through tanh to the feature-major output (host transposes).
  - PE warm-up MMs run during the input-DMA prologue to lift the HAM
    clock-gate to 2.4GHz before real work; A-tile DMA triggers alternate
    between the Sync and Scalar hardware DGE queues to halve trigger
    latency.
"""

import numpy as np

import concourse.bass as bass
import concourse.bacc as bacc
import concourse.tile as tile
from concourse import mybir
from concourse.bass import ts
from concourse.bass_utils import run_bass_kernel_spmd

F32 = mybir.dt.float32
BF16 = mybir.dt.bfloat16
FP8 = mybir.dt.float8e4
Alu = mybir.AluOpType
Act = mybir.ActivationFunctionType
DR = mybir.MatmulPerfMode.DoubleRow

B, N, IN_DIM, HID = 16, 4096, 64, 128
C = IN_DIM + HID              # 192
CB = C + 1                    # +1 ones row (bias folding)
M = 5
DEG = 16
N_CORES = 8
BL = B // N_CORES             # 2 batches per core
N_SUP = 2
NT = N // 128                 # 32 node tiles
NJP = NT // 2                 # 16 node-tile pairs (DoubleRow contraction)
NC4 = N // 4                  # 1024: i-chunk (4 PSUM groups live)
NM = 2 * N_SUP                # 4 sweep matrices: A_0, A_1, B_0, B_1

_prog_cache: dict = {}


def _install_ntff_hook():
    """Benchmark-only: wire up the NTFF profile hook that bass_utils
    expects under axon when trace=True (the antenv.axon_hooks shim module
    is absent in this image), and stub out the S3 artifact upload."""
    import sys
    import types

    try:
        import antenv
        import concourse.bass_utils as bu

        bu.upload_artifacts = lambda tmpdir: "local://" + tmpdir
        if "antenv.axon_hooks" in sys.modules:
            return
        import trn_agent_boot.trn_boot as tb

        hook = tb._ntff_profile_via_ctypes("/opt/axon/libaxon_pjrt.so")
        mod = types.ModuleType("antenv.axon_hooks")
        mod.get_axon_ntff_profile_hook = lambda: hook
        mod.set_axon_ntff_profile_hook = lambda h: None
        sys.modules["antenv.axon_hooks"] = mod
        antenv.axon_hooks = mod
    except Exception as e:  # profiling is best-effort
        print(f"ntff hook install failed: {e}")


def _build_program(n_sup: int):
    nc = bacc.Bacc(
        "TRN2",
        target_bir_lowering=False,
        debug=False,
        enable_asserts=False,
        num_devices=N_CORES,
    )
    nm = 2 * n_sup

    x0T_d = nc.dram_tensor("x0T", [BL, CB, N], BF16, kind="ExternalInput").ap()
    # fp8 DR-packed x_cat^T (c padded to 2x128) and Y-projection weights:
    # x8p[b, cc, kt, n] = x_cat^T[kt*128+cc, n]; wc8[cc, kt, col] likewise
    x8p_d = nc.dram_tensor("x8p", [BL, 128, 2, N], FP8, kind="ExternalInput").ap()
    wc8_d = nc.dram_tensor("wc8", [128, 2, 512], FP8, kind="ExternalInput").ap()
    # wc cols: 0:128 W0'+bias, 128:256 W1_0, 256:384 W1_1, 384:512 2*W2_0,
    # 512:640 2*W2_1
    wc_d = nc.dram_tensor("wc", [CB, M * HID], F32, kind="ExternalInput").ap()
    # ablk[j, p, m, k, i] = M_m[i, (2p+k)*128 + j], M = [A_0, A_1, B_0, B_1]
    ablk_d = nc.dram_tensor(
        "ablk", [128, NJP, nm, 2, N], FP8, kind="ExternalInput"
    ).ap()
    ident_d = nc.dram_tensor("ident", [128, 128], BF16, kind="ExternalInput").ap()
    # feature-major output: out[b, h, n]
    out_d = nc.dram_tensor("out", [BL, 128, N], F32, kind="ExternalOutput").ap()

    KCH = [(0, 128), (128, CB - 128)]   # C+1 split into partition chunks
    kn1 = CB - 128

    with tile.TileContext(nc) as tc:
        with (
            tc.tile_pool(name="persist", bufs=1) as persist,
            tc.tile_pool(name="apool", bufs=16) as apool,
            tc.tile_pool(name="stage", bufs=4) as stage,
            tc.tile_pool(name="psA", bufs=6, space="PSUM") as psA,
            tc.tile_pool(name="psT", bufs=2, space="PSUM") as psT,
        ):
            # ---------- identity + weights first, then PE warm-up ----------
            ident = persist.tile([128, 128], BF16, tag="ident")
            nc.sync.dma_start(out=ident[:], in_=ident_d[:, :])
            # early warm-up needs only ident (32KB DMA)
            for w in range(12):
                wp = psT.tile([128, 512], F32, tag="psT", name=f"warme{w}")
                nc.tensor.matmul(wp[:, 0:128], lhsT=ident[:], rhs=ident[:],
                                 start=True, stop=True)
            wc_bf = []
            for kc, (k0, kn) in enumerate(KCH):
                wst = stage.tile([128, M * HID], F32, tag="stage", name=f"wst{kc}")
                nc.sync.dma_start(out=wst[:kn, :], in_=wc_d[k0 : k0 + kn, :])
                wb = persist.tile([128, M * HID], BF16, tag=f"wc{kc}")
                nc.scalar.copy(out=wb[:kn, :], in_=wst[:kn, :])
                wc_bf.append(wb)
            # keep PE busy (and the HAM clock-gate warming) through the
            # rest of the input-DMA prologue
            for w in range(36):
                wp = psT.tile([128, 512], F32, tag="psT", name=f"warm{w}")
                nc.tensor.matmul(wp[:], lhsT=ident[:], rhs=wc_bf[0][:, 0:512],
                                 start=True, stop=True)
            # fp8 DR-packed projection operands
            wc8 = persist.tile([128, 2, 512], FP8, tag="wc8")
            nc.sync.dma_start(out=wc8[:], in_=wc8_d[:, :, :])

            # ---------- load x0T (bf16 for z0) + x8p (fp8 for Y projs) ----
            # x0T_bf[b]: [128, 8192]; cols [0:N] = c-chunk 0, [N:2N] = chunk 1
            x0T_bf, x8p = [], []
            for b in range(BL):
                x8b = persist.tile([128, 2, N], FP8, tag=f"x8_{b}", name=f"x8_{b}")
                nc.sync.dma_start(out=x8b[:], in_=x8p_d[b])
                x8p.append(x8b)
                xb = persist.tile([128, 2 * N], BF16, tag=f"xb{b}")
                for half in range(2):
                    sl = ts(half, N // 2)
                    nc.sync.dma_start(out=xb[:, sl], in_=x0T_d[b, 0:128, sl])
                    nc.sync.dma_start(
                        out=xb[:kn1, N + half * (N // 2) : N + (half + 1) * (N // 2)],
                        in_=x0T_d[b, 128:CB, sl],
                    )
                x0T_bf.append(xb)

            # ---------- stationaries ----------
            # yb[b][j, jb, m*128+h] = fp8 of [Y1_0 | Y1_1 | 2Y2_0 | 2Y2_1]
            yb = [persist.tile([128, NT, nm * HID], FP8, tag=f"yb{b}",
                               name=f"yb{b}") for b in range(BL)]

            # ---------- projections (one fp8 DoubleRow MM per tile) -------
            for b in range(BL):
                for t in range(NT):
                    p = psA.tile([128, 512], F32, tag="psA", name=f"yp{b}_{t}")
                    nc.tensor.matmul(
                        p[:],
                        lhsT=x8p[b][:, :, t * 128 : (t + 1) * 128],
                        rhs=wc8[:],
                        start=True,
                        stop=True,
                        perf_mode=DR,
                    )
                    if t & 1:
                        nc.vector.tensor_copy(out=yb[b][:, t, :], in_=p[:])
                    else:
                        nc.scalar.copy(out=yb[b][:, t, :], in_=p[:])

            # ---------- mega-pass: tanh(z0 + sum_m M_m @ Y_m) ----------
            # A-tile DMAs are issued PF steps ahead of their consuming MMs
            # (across sweep/chunk boundaries) so the first MM of a step never
            # waits on a just-triggered transfer.
            loads = [(chunk, m, p) for chunk in range(4)
                     for m in range(nm) for p in range(NJP)]
            tiles = {}

            def issue_load(idx):
                chunk, m, p = loads[idx]
                at = apool.tile([128, 2, NC4], FP8, tag="apool",
                                name=f"a_{chunk}_{m}_{p}")
                eng = nc.sync if (idx & 1) == 0 else nc.scalar
                eng.dma_start(
                    out=at[:],
                    in_=ablk_d[:, p, m, :, chunk * NC4 : (chunk + 1) * NC4],
                )
                tiles[idx] = at

            PF = 6
            for i in range(PF):
                issue_load(i)
            step = 0
            for chunk in range(4):
                psums = []
                for g in range(4):
                    isup, fc = g >> 1, g & 1
                    n0 = chunk * NC4 + isup * 512
                    pt = psA.tile([128, 512], F32, tag="psA",
                                  name=f"mp_{chunk}_{g}")
                    psums.append((pt, isup, fc, n0))
                    for kc, (k0, kn) in enumerate(KCH):
                        nc.tensor.matmul(
                            pt[:],
                            lhsT=wc_bf[kc][:kn, 0:128],
                            rhs=x0T_bf[fc][:kn, kc * N + n0 : kc * N + n0 + 512],
                            start=(kc == 0),
                            stop=False,
                        )
                for m in range(nm):
                    for p in range(NJP):
                        if step + PF < len(loads):
                            issue_load(step + PF)
                        at = tiles.pop(step)
                        for pt, isup, fc, n0 in psums:
                            nc.tensor.matmul(
                                pt[:],
                                lhsT=yb[fc][:, 2 * p : 2 * p + 2, ts(m, HID)],
                                rhs=at[:, :, isup * 512 : isup * 512 + 512],
                                start=False,
                                stop=(m == nm - 1 and p == NJP - 1),
                                perf_mode=DR,
                            )
                        step += 1
                for pt, isup, fc, n0 in psums:
                    # DVE copy frees the PSUM bank immediately (the scalar
                    # queue is backed up with DMA triggers); tanh runs later
                    # off SBUF.
                    st = stage.tile([128, M * HID], F32, tag="stage",
                                    name=f"st{fc}_{n0}")
                    nc.vector.tensor_copy(out=st[:, 0:512], in_=pt[:])
                    ot = stage.tile([128, M * HID], F32, tag="stage",
                                    name=f"ot{fc}_{n0}")
                    nc.scalar.activation(out=ot[:, 0:512], in_=st[:, 0:512],
                                         func=Act.Tanh)
                    nc.sync.dma_start(out=out_d[fc, :, n0 : n0 + 512], in_=ot[:, 0:512])

    nc.compile()
    return nc


def _build_ablk(sup_rows, sup_cols, sup_vals, n_sup):
    """Densify A_s and B_s = A_s^2 into the DoubleRow moving layout.

    ablk[j, p, m, k, i] = M_m[i, (2p+k)*128 + j]; fp8e4 exact for both.
    """
    import ml_dtypes
    from scipy import sparse

    out = np.empty((128, NJP, 2 * n_sup, 2, N), dtype=ml_dtypes.float8_e4m3)
    for s in range(n_sup):
        a_sp = sparse.csr_matrix(
            (
                sup_vals[s].astype(np.float32),
                (sup_rows[s].astype(np.int64), sup_cols[s].astype(np.int64)),
            ),
            shape=(N, N),
        )
        a = a_sp.toarray()
        b2 = (a_sp @ a_sp).toarray()
        for m, mat in ((s, a), (n_sup + s, b2)):
            # at[col, row] layout -> [j, p, k, i]
            out[:, :, m] = (
                mat.T.reshape(NJP, 2, 128, N).transpose(2, 0, 1, 3)
                .astype(ml_dtypes.float8_e4m3)
            )
    return out


def _prep_core_inputs(inputs, state_t, weights, biases, sup_rows, sup_cols, sup_vals):
    """Host-side sharding: batch-parallel slices + layout prep."""
    import ml_dtypes

    w5 = weights.reshape(C, M, HID)
    wc = np.zeros((CB, M, HID), dtype=np.float32)
    wc[:C, 0] = w5[:, 0] - w5[:, 2] - w5[:, 4]
    wc[C, 0] = biases.astype(np.float32)          # bias via ones row
    wc[:C, 1] = w5[:, 1]                          # W1_0
    wc[:C, 2] = w5[:, 3]                          # W1_1
    wc[:C, 3] = 2.0 * w5[:, 2]                    # 2*W2_0
    wc[:C, 4] = 2.0 * w5[:, 4]                    # 2*W2_1
    wc = np.ascontiguousarray(wc.reshape(CB, M * HID))

    ablk = _build_ablk(sup_rows, sup_cols, sup_vals, N_SUP)
    ident = np.eye(128, dtype=ml_dtypes.bfloat16)
    # wc8[cc, kt, col] = wc[kt*128+cc, 128+col] (fp8, c padded to 256)
    wcp = np.zeros((2, 128, 512), dtype=np.float32)
    wcp[0] = wc[0:128, 128:640]
    wcp[1, : CB - 128] = wc[128:CB, 128:640]
    wc8 = np.ascontiguousarray(wcp.transpose(1, 0, 2)).astype(ml_dtypes.float8_e4m3)

    in_maps = []
    for core in range(N_CORES):
        b0 = core * BL
        xcat = np.concatenate(
            [
                inputs[b0 : b0 + BL],
                state_t[b0 : b0 + BL],
                np.ones((BL, N, 1), dtype=np.float32),
            ],
            axis=2,
        )  # [BL, N, CB]
        x0T = np.ascontiguousarray(xcat.transpose(0, 2, 1)).astype(ml_dtypes.bfloat16)
        # x8p[b, cc, kt, n] = x_cat^T[kt*128+cc, n] (fp8, c padded to 256)
        xp = np.zeros((BL, 2, 128, N), dtype=np.float32)
        xp[:, 0] = xcat.transpose(0, 2, 1)[:, 0:128]
        xp[:, 1, : CB - 128] = xcat.transpose(0, 2, 1)[:, 128:CB]
        x8p = np.ascontiguousarray(xp.transpose(0, 2, 1, 3)).astype(
            ml_dtypes.float8_e4m3
        )
        in_maps.append(
            {"x0T": x0T, "wc": wc, "ablk": ablk, "ident": ident,
             "x8p": x8p, "wc8": wc8}
        )
    return in_maps


def kernel(
    inputs,
    state_t,
    weights,
    biases,
    sup_rows,
    sup_cols,
    sup_vals,
    _bench=None,
):
    inputs = np.asarray(inputs)
    state_t = np.asarray(state_t)
    weights = np.asarray(weights, dtype=np.float32)
    biases = np.asarray(biases, dtype=np.float32)
    sup_rows = np.asarray(sup_rows)
    sup_cols = np.asarray(sup_cols)
    sup_vals = np.asarray(sup_vals)

    if "prog" not in _prog_cache:
        _prog_cache["prog"] = _build_program(N_SUP)
    nc = _prog_cache["prog"]

    in_maps = _prep_core_inputs(
        inputs, state_t, weights, biases, sup_rows, sup_cols, sup_vals
    )
    trace = _bench is not None
    if trace:
        _install_ntff_hook()
    res = run_bass_kernel_spmd(nc, in_maps, list(range(N_CORES)), trace=trace)
    if _bench is not None:
        _bench["exec_time_ns"] = res.exec_time_ns
        _bench["mean_exec_time_ns"] = res.mean_exec_time_ns
        _bench["results"] = res

    out = np.empty((B, N, HID), dtype=np.float32)
    for core in range(N_CORES):
        o = res.results[core]["out"]  # [BL, 128, N] feature-major
        for b in range(BL):
            out[core * BL + b] = np.asarray(o[b], dtype=np.float32).T
    return out
